# revision 1
# baseline (speedup 1.0000x reference)
"""Trainium2 Bass kernel for nn_MultiHeadAttention (B=8192, D=1024, 16 heads
used only via the softmax scale 1/8).

Strategy (8 NeuronCores, zero inter-core communication):
  - Rows (batch axis) of the attention output are sharded: core c owns rows
    [c*1024, (c+1)*1024).
  - Every core recomputes the full K^T and V projections for all 8192 rows
    (replicated compute instead of an all-gather; collectives on this part
    are slower than the 2x17 GFLOP of extra matmul).
  - Attention runs in a transposed-energy ("E^T") layout so no probability
    transpose is ever needed:
        E^T[j, i] = sum_o K^T[o, j] * Q^T[o, i]
        P^T = exp(E^T * 0.125)            (no max subtraction; |E|<40, safe)
        out_unnorm[i, o] = sum_j P^T[j, i] * V[j, o]
        s[i] = sum_j P^T[j, i]            (matmul against a ones vector)
        out = out_unnorm / s + bv         (bv folded in post-normalization)
  - All big matmuls run in float32r (full-rate streaming on the PE at
    N=512) with fp32 PSUM accumulation.
"""

import sys

sys.path.insert(0, "/opt/trn_rl_repo")

import numpy as np

import concourse.bass as bass  # noqa: F401
import concourse.tile as tile
from concourse import bacc, mybir
from concourse.bass_utils import run_bass_kernel_spmd
from concourse.masks import make_identity

B = 8192
D = 1024
P = 128
NCORES = 8
R = B // NCORES  # 1024 rows per core
JBLK = 512  # j-block (keys/values) streamed per iteration
NJB = B // JBLK  # 16
DO = D // P  # 8 feature chunks of 128
IC = R // P  # 8 row chunks of 128 per core
F32 = mybir.dt.float32
F32R = mybir.dt.float32r
BF16 = mybir.dt.bfloat16
AF = mybir.ActivationFunctionType
ALU = mybir.AluOpType
SCALE = 0.125  # 1/sqrt(head_dim=64)




def _transpose_rows_to_sbuf(nc, tp_psum, identity, row_sb, xt_dst, jj):
    """PE-transpose a [128, 1024] natural-layout row chunk into the
    [128(d_in), DO, ...] transposed SBUF tile at free offset jj*128."""
    for dd in range(DO):
        tp = tp_psum.tile([P, P], F32, tag="tp")
        nc.tensor.transpose(tp, row_sb[:, dd * P : (dd + 1) * P], identity)
        nc.vector.tensor_copy(
            out=xt_dst[:, dd, jj * P : (jj + 1) * P], in_=tp
        )


def build_program():
    nc = bacc.Bacc(
        "TRN2", target_bir_lowering=False, debug=False, num_devices=NCORES
    )
    x = nc.dram_tensor("x", [B, D], F32, kind="ExternalInput").ap()
    x_loc = nc.dram_tensor("x_loc", [R, D], F32, kind="ExternalInput").ap()
    w_q = nc.dram_tensor("Wq", [D, D], F32, kind="ExternalInput").ap()
    w_k = nc.dram_tensor("Wk", [D, D], F32, kind="ExternalInput").ap()
    w_v = nc.dram_tensor("Wv", [D, D], F32, kind="ExternalInput").ap()
    b_q = nc.dram_tensor("bq", [D], F32, kind="ExternalInput").ap()
    b_k = nc.dram_tensor("bk", [D], F32, kind="ExternalInput").ap()
    b_v = nc.dram_tensor("bv", [D], F32, kind="ExternalInput").ap()
    out_loc = nc.dram_tensor("out_loc", [R, D], F32, kind="ExternalOutput").ap()

    with tile.TileContext(nc) as tc:
        _body(nc, tc, x, x_loc, w_q, w_k, w_v, b_q, b_k, b_v, out_loc)
    nc.compile()
    return nc


def _body(nc, tc, x, x_loc, w_q, w_k, w_v, b_q, b_k, b_v, out_loc):
    from contextlib import ExitStack

    outer = ExitStack()
    outer.__enter__()
    # ---- persistent pools (whole kernel) ----
    const_pool = outer.enter_context(tc.tile_pool(name="const", bufs=1))
    identity = const_pool.tile([P, P], F32)
    make_identity(nc, identity)
    ones_f32 = const_pool.tile([P, 2], F32)
    nc.vector.memset(ones_f32, 1.0)
    ones = const_pool.tile([P, 2], BF16)
    nc.vector.tensor_copy(out=ones, in_=ones_f32)
    bq_sb = const_pool.tile([P, DO], F32)
    nc.sync.dma_start(bq_sb, b_q.rearrange("(oo p) -> p oo", p=P))
    bk_sb = const_pool.tile([P, DO], F32)
    nc.sync.dma_start(bk_sb, b_k.rearrange("(oo p) -> p oo", p=P))
    ones_row = const_pool.tile([1, P], F32)
    nc.vector.memset(ones_row, 1.0)
    # broadcast bv across all 128 partitions with a K=1 matmul:
    # load bv into partition 0 of bv_bc, then out[p, o] = 1 * bv[o]
    bv_bc = const_pool.tile([P, D], F32)
    nc.sync.dma_start(bv_bc[0:1, :], b_v[None, :])
    with tc.tile_pool(name="bv_psum", bufs=2, space="PSUM") as bvp:
        for oh in range(2):
            pt = bvp.tile([P, 512], F32, tag="bvp")
            nc.tensor.matmul(
                pt,
                ones_row,
                bv_bc[0:1, oh * 512 : (oh + 1) * 512],
                start=True,
                stop=True,
            )
            nc.vector.tensor_copy(out=bv_bc[:, oh * 512 : (oh + 1) * 512], in_=pt)

    qt_pool = outer.enter_context(tc.tile_pool(name="qt", bufs=1))
    qt = qt_pool.tile([P, DO, R], F32R)  # Q^T: [o_in, o_out, i]  (4 MB)

    sums_pool = outer.enter_context(tc.tile_pool(name="sums", bufs=1))
    sums_acc = sums_pool.tile([P, 2 * IC], F32)  # per-row exp-sums (even cols)
    rsum = sums_pool.tile([P, 2 * IC], F32)

    # DRAM scratch for the full K^T / V (32 MB each) — streamed in phase 2.
    dram = outer.enter_context(tc.tile_pool(name="dram", bufs=1, space="DRAM"))
    kt_dram = dram.tile([DO, P, B], F32R)  # K^T: [o_out][o_in][j]
    v_dram = dram.tile([B, D], BF16)  # V: natural [j, o]

    # =========================================================
    # Phase 0+1: weight transposes, Q^T (local), K^T/V (full)
    # =========================================================
    with ExitStack() as p1:
        wt_pool = p1.enter_context(tc.tile_pool(name="wt", bufs=1))
        wqt = wt_pool.tile([P, DO, D], F32R)  # W^T: [d_in, d_out, o] (4 MB)
        wkt = wt_pool.tile([P, DO, D], F32R)
        wvt = wt_pool.tile([P, DO, D], BF16)

        row_pool = p1.enter_context(tc.tile_pool(name="rows", bufs=2))
        xt_pool = p1.enter_context(tc.tile_pool(name="xt", bufs=2))
        st_pool = p1.enter_context(tc.tile_pool(name="stage", bufs=2))
        tp_psum = p1.enter_context(tc.tile_pool(name="tp_ps", bufs=2, space="PSUM"))
        mm_psum = p1.enter_context(tc.tile_pool(name="mm_ps", bufs=4, space="PSUM"))

        # -- transpose the three weight matrices into SBUF --
        for wt_sb, w_dram in ((wqt, w_q), (wkt, w_k), (wvt, w_v)):
            for oo in range(DO):
                wrow = row_pool.tile([P, D], F32, tag="row")
                nc.sync.dma_start(wrow, w_dram[oo * P : (oo + 1) * P, :])
                for dd in range(DO):
                    tp = tp_psum.tile([P, P], F32, tag="tp")
                    nc.tensor.transpose(
                        tp, wrow[:, dd * P : (dd + 1) * P], identity
                    )
                    nc.vector.tensor_copy(
                        out=wt_sb[:, dd, oo * P : (oo + 1) * P], in_=tp
                    )

        # -- Q^T for the local shard, in halves of 512 rows --
        for ih in range(R // JBLK):
            xt_blk = xt_pool.tile([P, DO, JBLK], F32R, tag="xt")
            for jj in range(JBLK // P):
                xrow = row_pool.tile([P, D], F32, tag="row")
                nc.sync.dma_start(
                    xrow, x_loc[(ih * 4 + jj) * P : (ih * 4 + jj + 1) * P, :]
                )
                _transpose_rows_to_sbuf(nc, tp_psum, identity, xrow, xt_blk, jj)
            for oo in range(DO):
                pq = mm_psum.tile([P, JBLK], F32, tag="mm")
                for dd in range(DO):
                    nc.tensor.matmul(
                        pq,
                        (wqt[:, dd, oo * P : (oo + 1) * P]),
                        (xt_blk[:, dd, :]),
                        start=(dd == 0),
                        stop=(dd == DO - 1),
                    )
                nc.scalar.activation(
                    qt[:, oo, ih * JBLK : (ih + 1) * JBLK],
                    pq,
                    AF.Identity,
                    bias=bq_sb[:, oo : oo + 1],
                )

        # -- full K^T and V, streamed over 16 j-blocks of 512 rows --
        for jb in range(NJB):
            xt_blk = xt_pool.tile([P, DO, JBLK], F32R, tag="xt")
            for jj in range(JBLK // P):
                xrow = row_pool.tile([P, D], F32, tag="row")
                nc.sync.dma_start(
                    xrow, x[(jb * 4 + jj) * P : (jb * 4 + jj + 1) * P, :]
                )
                _transpose_rows_to_sbuf(nc, tp_psum, identity, xrow, xt_blk, jj)
            xt_bf = xt_pool.tile([P, DO, JBLK], BF16, tag="xtb")
            nc.vector.tensor_copy(out=xt_bf, in_=xt_blk)
            # K^T block: [o, j]
            for oo in range(DO):
                pk = mm_psum.tile([P, JBLK], F32, tag="mm")
                for dd in range(DO):
                    nc.tensor.matmul(
                        pk,
                        (wkt[:, dd, oo * P : (oo + 1) * P]),
                        (xt_blk[:, dd, :]),
                        start=(dd == 0),
                        stop=(dd == DO - 1),
                    )
                kst = st_pool.tile([P, JBLK], F32R, tag="kst")
                nc.scalar.activation(
                    kst, pk, AF.Identity, bias=bk_sb[:, oo : oo + 1]
                )
                nc.sync.dma_start(
                    kt_dram[oo, :, jb * JBLK : (jb + 1) * JBLK], kst
                )
            # V block: natural [j, o], bias deferred to the epilogue
            for jj in range(JBLK // P):
                vst = st_pool.tile([P, D], BF16, tag="vst")
                pv_h = [mm_psum.tile([P, JBLK], F32, tag="mm", name="pv") for _ in range(2)]
                for dd in range(DO):
                    for oh in range(2):
                        nc.tensor.matmul(
                            pv_h[oh],
                            (xt_bf[:, dd, jj * P : (jj + 1) * P]),
                            (wvt[:, dd, oh * 512 : (oh + 1) * 512]),
                            start=(dd == 0),
                            stop=(dd == DO - 1),
                        )
                for oh in range(2):
                    nc.vector.tensor_copy(
                        out=vst[:, oh * 512 : (oh + 1) * 512], in_=pv_h[oh]
                    )
                nc.sync.dma_start(
                    v_dram[jb * JBLK + jj * P : jb * JBLK + (jj + 1) * P, :], vst
                )

    # =========================================================
    # Phase 2: streamed attention in E^T layout
    # =========================================================
    with ExitStack() as p2:
        oa_pool = p2.enter_context(tc.tile_pool(name="oacc", bufs=1))
        outacc = oa_pool.tile([P, IC, D], F32)  # 4 MB

        kt_pool = p2.enter_context(tc.tile_pool(name="ktb", bufs=3))
        v_pool = p2.enter_context(tc.tile_pool(name="vtb", bufs=3))
        pt_pool = p2.enter_context(tc.tile_pool(name="ptb", bufs=3))
        e_psum = p2.enter_context(tc.tile_pool(name="e_ps", bufs=4, space="PSUM"))
        o_psum = p2.enter_context(tc.tile_pool(name="o_ps", bufs=3, space="PSUM"))
        s_psum = p2.enter_context(tc.tile_pool(name="s_ps", bufs=1, space="PSUM"))

        for jb in range(NJB):
            ktb = kt_pool.tile([P, DO, JBLK], F32R, tag="ktb")
            for oo in range(DO):
                nc.sync.dma_start(
                    ktb[:, oo, :], kt_dram[oo, :, jb * JBLK : (jb + 1) * JBLK]
                )
            vtb = v_pool.tile([P, JBLK // P, D], BF16, tag="vtb")
            nc.sync.dma_start(
                vtb,
                v_dram[jb * JBLK : (jb + 1) * JBLK, :].rearrange(
                    "(jj p) o -> p jj o", p=P
                ),
            )
            # unnormalized probabilities P^T for this j-block: [j, i]
            ptb = pt_pool.tile([P, JBLK // P, R], BF16, tag="ptb")
            for jj in range(JBLK // P):
                pe_h = [
                    e_psum.tile([P, JBLK], F32, tag="pe", name="pe")
                    for _ in range(R // JBLK)
                ]
                for oo in range(DO):
                    for ih in range(R // JBLK):
                        nc.tensor.matmul(
                            pe_h[ih],
                            (ktb[:, oo, jj * P : (jj + 1) * P]),
                            (qt[:, oo, ih * JBLK : (ih + 1) * JBLK]),
                            start=(oo == 0),
                            stop=(oo == DO - 1),
                        )
                for ih in range(R // JBLK):
                    nc.scalar.activation(
                        ptb[:, jj, ih * JBLK : (ih + 1) * JBLK],
                        pe_h[ih],
                        AF.Exp,
                        scale=SCALE,
                    )
            # row sums of P^T (reduce over j): matmul against ones
            # out_unnorm += P^T.T @ V, with the exp-sums matmul sharing each
            # stationary ptb tile (3 streams per weight load)
            ps = s_psum.tile([P, 2 * IC], F32, tag="ps")
            for ic in range(IC):
                po_h = [o_psum.tile([P, 512], F32, tag="po", name="po") for _ in range(2)]
                for jj in range(JBLK // P):
                    for oh in range(2):
                        nc.tensor.matmul(
                            po_h[oh],
                            (ptb[:, jj, ic * P : (ic + 1) * P]),
                            (vtb[:, jj, oh * 512 : (oh + 1) * 512]),
                            start=(jj == 0),
                            stop=(jj == JBLK // P - 1),
                        )
                    nc.tensor.matmul(
                        ps[:, 2 * ic : 2 * ic + 2],
                        (ptb[:, jj, ic * P : (ic + 1) * P]),
                        (ones),
                        start=(ic == 0 and jj == 0),
                        stop=(ic == IC - 1 and jj == JBLK // P - 1),
                    )
                for oh in range(2):
                    dst = outacc[:, ic, oh * 512 : (oh + 1) * 512]
                    if jb == 0:
                        nc.vector.tensor_copy(out=dst, in_=po_h[oh])
                    else:
                        nc.vector.tensor_tensor(dst, po_h[oh], dst, ALU.add)
            if jb == 0:
                nc.vector.tensor_copy(out=sums_acc, in_=ps)
            else:
                nc.vector.tensor_tensor(sums_acc, ps, sums_acc, ALU.add)

        # ---- epilogue: normalize, add bv, write out ----
        nc.vector.reciprocal(rsum, sums_acc)
        fin_pool = p2.enter_context(tc.tile_pool(name="fin", bufs=2))
        for ic in range(IC):
            ofin = fin_pool.tile([P, D], F32, tag="ofin")
            nc.vector.tensor_scalar_mul(ofin, outacc[:, ic, :], rsum[:, 2 * ic : 2 * ic + 1])
            nc.vector.tensor_tensor(ofin, ofin, bv_bc, ALU.add)
            nc.sync.dma_start(out_loc[ic * P : (ic + 1) * P, :], ofin)

    outer.close()


_NC_CACHE = None


def _get_program():
    global _NC_CACHE
    if _NC_CACHE is None:
        _NC_CACHE = build_program()
    return _NC_CACHE


def _run(inputs, trace=False):
    nc = _get_program()
    x = np.ascontiguousarray(np.asarray(inputs["x"], dtype=np.float32))
    common = {
        k: np.ascontiguousarray(np.asarray(inputs[k], dtype=np.float32))
        for k in ("Wq", "Wk", "Wv", "bq", "bk", "bv")
    }
    in_maps = [
        {"x": x, "x_loc": np.ascontiguousarray(x[c * R : (c + 1) * R]), **common}
        for c in range(NCORES)
    ]
    res = run_bass_kernel_spmd(
        nc, in_maps, core_ids=list(range(NCORES)), trace=trace
    )
    out = np.concatenate([res.results[c]["out_loc"] for c in range(NCORES)], axis=0)
    return out.reshape(B, D, 1).astype(np.float32), res


def kernel(**inputs):
    out, _ = _run(inputs, trace=False)
    return out



# revision 16
# speedup vs baseline: 1.5659x; 1.5659x over previous
"""Trainium2 Bass kernel for nn_MultiHeadAttention (B=8192, D=1024, 16 heads
used only via the softmax scale 1/8).

Strategy (8 NeuronCores, row-sharded attention + AllGather collectives):
  - Rows (batch axis) of the attention output are sharded: core c owns rows
    [c*1024, (c+1)*1024).
  - Algebraic restructuring removes the K projection and all Q/K weight
    transposes:
        E[i, j] = Q_i . K_j = (Wk^T Q_i) . x_j + (Q_i . bk)
    The per-row constant Q_i.bk cancels in softmax, so with
        M' = Wq^T Wk          (from natural-layout weights, no transposes)
        Z^T = M'^T x^T + (Wk^T bq)  (per-core, local rows only)
    the energy is E^T[j, i] = sum_d x^T[d, j] * Z^T[d, i].
  - Each core transposes only its local 1024 rows of x; the full x^T and the
    full V (bf16) are assembled with two AllGather collectives that run on
    the TOPSP/SDMA hardware, fully overlapped with the projection matmuls.
  - Attention runs in the transposed-energy ("E^T") layout so no probability
    transpose is needed:
        P^T = exp(E^T * 0.125)           (no max subtraction; |logit| small)
        out_unnorm[i, o] = sum_j P^T[j, i] * V[j, o]
        s[i] = sum_j P^T[j, i]           (matmul against a ones vector)
        out = out_unnorm / s + bv        (bv folded in post-normalization)
  - Big matmuls run in float32r (full-rate streaming at N=512) with fp32
    PSUM accumulation; P/V use bf16.
"""

import sys

sys.path.insert(0, "/opt/trn_rl_repo")

import numpy as np

import concourse.bass as bass  # noqa: F401
import concourse.tile as tile
from concourse import bacc, mybir
from concourse.bass_utils import run_bass_kernel_spmd
from concourse.masks import make_identity

B = 8192
D = 1024
P = 128
NCORES = 8
R = B // NCORES  # 1024 rows per core
JBLK = 512  # j-block (keys/values) streamed per iteration
NJB = B // JBLK  # 16
DO = D // P  # 8 feature chunks of 128
IC = R // P  # 8 row chunks of 128 per core
F32 = mybir.dt.float32
F32R = mybir.dt.float32r
BF16 = mybir.dt.bfloat16
AF = mybir.ActivationFunctionType
ALU = mybir.AluOpType
SCALE = 0.125  # 1/sqrt(head_dim=64)


def build_program():
    nc = bacc.Bacc(
        "TRN2", target_bir_lowering=False, debug=False, num_devices=NCORES
    )
    x_loc = nc.dram_tensor("x_loc", [R, D], F32, kind="ExternalInput").ap()
    w_q = nc.dram_tensor("Wq", [D, D], F32, kind="ExternalInput").ap()
    w_k = nc.dram_tensor("Wk", [D, D], F32, kind="ExternalInput").ap()
    w_v = nc.dram_tensor("Wv", [D, D], F32, kind="ExternalInput").ap()
    b_q = nc.dram_tensor("bq", [D], F32, kind="ExternalInput").ap()
    b_v = nc.dram_tensor("bv", [D], F32, kind="ExternalInput").ap()
    out_loc = nc.dram_tensor("out_loc", [R, D], F32, kind="ExternalOutput").ap()

    with tile.TileContext(nc) as tc:
        _body(nc, tc, x_loc, w_q, w_k, w_v, b_q, b_v, out_loc)
    nc.compile()
    return nc


def _body(nc, tc, x_loc, w_q, w_k, w_v, b_q, b_v, out_loc):
    from contextlib import ExitStack

    outer = ExitStack()
    outer.__enter__()
    # ---- persistent pools (whole kernel) ----
    const_pool = outer.enter_context(tc.tile_pool(name="const", bufs=1))
    identity = const_pool.tile([P, P], F32)
    make_identity(nc, identity)
    ones_f32 = const_pool.tile([P, 2], F32)
    nc.vector.memset(ones_f32, 1.0)
    ones = const_pool.tile([P, 2], BF16)
    nc.vector.tensor_copy(out=ones, in_=ones_f32)
    bq_sb = const_pool.tile([P, DO], F32R)
    nc.sync.dma_start(bq_sb, b_q.rearrange("(oo p) -> p oo", p=P).bitcast(F32R))
    ones_row = const_pool.tile([1, P], F32)
    nc.vector.memset(ones_row, 1.0)
    ones512 = const_pool.tile([1, JBLK], F32)
    nc.vector.memset(ones512, 1.0)
    # broadcast bv across all 128 partitions with a K=1 matmul:
    bv_bc = const_pool.tile([P, D], F32)
    nc.sync.dma_start(bv_bc[0:1, :], b_v[None, :])
    with tc.tile_pool(name="bv_psum", bufs=2, space="PSUM") as bvp:
        for oh in range(2):
            pt = bvp.tile([P, 512], F32, tag="bvp")
            nc.tensor.matmul(
                pt,
                ones_row,
                bv_bc[0:1, oh * 512 : (oh + 1) * 512],
                start=True,
                stop=True,
            )
            nc.vector.tensor_copy(out=bv_bc[:, oh * 512 : (oh + 1) * 512], in_=pt)

    zt_pool = outer.enter_context(tc.tile_pool(name="zt", bufs=1))
    zt = zt_pool.tile([P, DO, R], F32R)  # Z^T: [d_in, dd, i]  (4 MB)

    sums_pool = outer.enter_context(tc.tile_pool(name="sums", bufs=1))
    sums_acc = sums_pool.tile([P, 2 * IC], F32)  # per-row exp-sums (even cols)
    rsum = sums_pool.tile([P, 2 * IC], F32)

    # DRAM scratch: local x^T / V shards + AllGather outputs (Shared).
    dram = outer.enter_context(tc.tile_pool(name="dram", bufs=1, space="DRAM"))
    xt_loc_d = dram.tile([DO, P, R], F32R)  # local x^T shard (4 MB)
    xt_g = dram.tile([NCORES, DO, P, R], F32R, addr_space="Shared")  # 32 MB
    v_loc_d = dram.tile([R, D], BF16)  # local V shard (2 MB)
    v_g = dram.tile([NCORES, R, D], BF16, addr_space="Shared")  # 16 MB

    # =========================================================
    # Phase 1: x^T (local), AG(x^T); M' = Wq^T Wk, Z^T; V, AG(V)
    # =========================================================
    with ExitStack() as p1:
        row_pool = p1.enter_context(tc.tile_pool(name="rows", bufs=2))
        tp_psum = p1.enter_context(tc.tile_pool(name="tp_ps", bufs=2, space="PSUM"))
        mm_psum = p1.enter_context(tc.tile_pool(name="mm_ps", bufs=4, space="PSUM"))
        g_psum = p1.enter_context(tc.tile_pool(name="g_ps", bufs=1, space="PSUM"))
        st_pool = p1.enter_context(tc.tile_pool(name="stage", bufs=2))

        xt_pool = p1.enter_context(tc.tile_pool(name="xt", bufs=1))
        xt = xt_pool.tile([P, DO, R], F32R)  # x_loc^T (4 MB)
        xt_bf = xt_pool.tile([P, DO, R], BF16)  # bf16 copy for V stationary

        # -- transpose local x rows into x^T; ship shard to DRAM + AllGather --
        for ic in range(IC):
            xrow = row_pool.tile([P, D], F32, tag="row")
            nc.sync.dma_start(xrow, x_loc[ic * P : (ic + 1) * P, :])
            for dd in range(DO):
                tp = tp_psum.tile([P, P], F32, tag="tp")
                nc.tensor.transpose(tp, xrow[:, dd * P : (dd + 1) * P], identity)
                nc.vector.tensor_copy(
                    out=xt[:, dd, ic * P : (ic + 1) * P], in_=tp
                )
        nc.vector.tensor_copy(out=xt_bf, in_=xt)
        for dd in range(DO):
            nc.sync.dma_start(xt_loc_d[dd], xt[:, dd, :])
        nc.gpsimd.collective_compute(
            "AllGather",
            mybir.AluOpType.bypass,
            replica_groups=[list(range(NCORES))],
            ins=[xt_loc_d.opt()],
            outs=[xt_g.opt()],
        )

        # -- M' = Wq^T Wk from natural-layout weights (no transposes) --
        mp_pool = p1.enter_context(tc.tile_pool(name="mp", bufs=1))
        mp = mp_pool.tile([P, DO, D], F32R)  # M'[d', dp, d] (4 MB)
        g_row = const_pool.tile([1, D], F32R)  # g = Wk^T bq as a row
        with ExitStack() as wqk:
            wq_pool = wqk.enter_context(tc.tile_pool(name="wq", bufs=1))
            wq_sb = wq_pool.tile([P, DO, D], F32R)  # Wq rows: [o, oo, d']
            wk_pool = wqk.enter_context(tc.tile_pool(name="wk", bufs=1))
            wk_sb = wk_pool.tile([P, DO, D], F32R)  # Wk rows: [o, oo, d]
            nc.sync.dma_start(
                wq_sb, w_q.rearrange("(oo p) d -> p oo d", p=P).bitcast(F32R)
            )
            nc.sync.dma_start(
                wk_sb, w_k.rearrange("(oo p) d -> p oo d", p=P).bitcast(F32R)
            )
            for dp in range(DO):
                for dh in range(2):
                    pm = mm_psum.tile([P, 512], F32, tag="mm")
                    for oo in range(DO):
                        nc.tensor.matmul(
                            pm,
                            wq_sb[:, oo, dp * P : (dp + 1) * P],
                            wk_sb[:, oo, dh * 512 : (dh + 1) * 512],
                            start=(oo == 0),
                            stop=(oo == DO - 1),
                        )
                    nc.vector.tensor_copy(
                        out=mp[:, dp, dh * 512 : (dh + 1) * 512], in_=pm
                    )
            # -- g = Wk^T bq as a row vector (wide matmuls only) --
            for dh in range(2):
                pg = g_psum.tile([1, JBLK], F32, tag="g")
                for oo in range(DO):
                    nc.tensor.matmul(
                        pg,
                        bq_sb[:, oo : oo + 1],
                        wk_sb[:, oo, dh * 512 : (dh + 1) * 512],
                        start=(oo == 0),
                        stop=(oo == DO - 1),
                    )
                nc.vector.tensor_copy(
                    out=g_row[:, dh * 512 : (dh + 1) * 512], in_=pg
                )

        # -- Z^T = M'^T x^T + g --
        for dd in range(DO):
            for ih in range(R // JBLK):
                pz = mm_psum.tile([P, JBLK], F32, tag="mm")
                for dp in range(DO):
                    nc.tensor.matmul(
                        pz,
                        mp[:, dp, dd * P : (dd + 1) * P],
                        xt[:, dp, ih * JBLK : (ih + 1) * JBLK],
                        start=(dp == 0),
                        stop=False,
                    )
                # += g[d] * 1[i]  (K=1 outer product adds the bq contribution)
                nc.tensor.matmul(
                    pz,
                    g_row[:, dd * P : (dd + 1) * P],
                    ones512.bitcast(F32R),
                    start=False,
                    stop=True,
                )
                nc.scalar.activation(
                    zt[:, dd, ih * JBLK : (ih + 1) * JBLK],
                    pz,
                    AF.Identity,
                )

        # -- V local (natural [j, o]; bias bv deferred to epilogue), AG(V) --
        wv_pool = p1.enter_context(tc.tile_pool(name="wv", bufs=1))
        wvt = wv_pool.tile([P, DO, D], BF16)  # Wv^T: [d, dd, o] (2 MB)
        for oo in range(DO):
            wrow = row_pool.tile([P, D], F32, tag="row")
            nc.sync.dma_start(wrow, w_v[oo * P : (oo + 1) * P, :])
            for dd in range(DO):
                tp = tp_psum.tile([P, P], F32, tag="tp")
                nc.tensor.transpose(tp, wrow[:, dd * P : (dd + 1) * P], identity)
                nc.vector.tensor_copy(
                    out=wvt[:, dd, oo * P : (oo + 1) * P], in_=tp
                )
        for jj in range(IC):
            vst = st_pool.tile([P, D], BF16, tag="vst")
            pv_h = [
                mm_psum.tile([P, 512], F32, tag="mm", name="pv") for _ in range(2)
            ]
            for dd in range(DO):
                for oh in range(2):
                    nc.tensor.matmul(
                        pv_h[oh],
                        xt_bf[:, dd, jj * P : (jj + 1) * P],
                        wvt[:, dd, oh * 512 : (oh + 1) * 512],
                        start=(dd == 0),
                        stop=(dd == DO - 1),
                    )
            for oh in range(2):
                nc.vector.tensor_copy(
                    out=vst[:, oh * 512 : (oh + 1) * 512], in_=pv_h[oh]
                )
            nc.sync.dma_start(v_loc_d[jj * P : (jj + 1) * P, :], vst)
        nc.gpsimd.collective_compute(
            "AllGather",
            mybir.AluOpType.bypass,
            replica_groups=[list(range(NCORES))],
            ins=[v_loc_d.opt()],
            outs=[v_g.opt()],
        )

    # =========================================================
    # Phase 2: streamed attention in E^T layout
    # =========================================================
    with ExitStack() as p2:
        oa_pool = p2.enter_context(tc.tile_pool(name="oacc", bufs=1))
        outacc = oa_pool.tile([P, IC, D], F32)  # 4 MB

        xtb_pool = p2.enter_context(tc.tile_pool(name="xtb", bufs=3))
        v_pool = p2.enter_context(tc.tile_pool(name="vtb", bufs=3))
        pt_pool = p2.enter_context(tc.tile_pool(name="ptb", bufs=3))
        e_psum = p2.enter_context(tc.tile_pool(name="e_ps", bufs=4, space="PSUM"))
        o_psum = p2.enter_context(tc.tile_pool(name="o_ps", bufs=3, space="PSUM"))
        s_psum = p2.enter_context(tc.tile_pool(name="s_ps", bufs=1, space="PSUM"))

        for jb in range(NJB):
            rank, half = jb // 2, jb % 2
            xtb = xtb_pool.tile([P, DO, JBLK], F32R, tag="xtb")
            for dd in range(DO):
                nc.sync.dma_start(
                    xtb[:, dd, :],
                    xt_g[rank, dd, :, half * JBLK : (half + 1) * JBLK],
                )
            vtb = v_pool.tile([P, JBLK // P, D], BF16, tag="vtb")
            nc.sync.dma_start(
                vtb,
                v_g[rank, half * JBLK : (half + 1) * JBLK, :].rearrange(
                    "(jj p) o -> p jj o", p=P
                ),
            )
            # unnormalized probabilities P^T for this j-block: [j, i]
            ptb = pt_pool.tile([P, JBLK // P, R], BF16, tag="ptb")
            for jj in range(JBLK // P):
                pe_h = [
                    e_psum.tile([P, JBLK], F32, tag="pe", name="pe")
                    for _ in range(R // JBLK)
                ]
                for dd in range(DO):
                    for ih in range(R // JBLK):
                        nc.tensor.matmul(
                            pe_h[ih],
                            xtb[:, dd, jj * P : (jj + 1) * P],
                            zt[:, dd, ih * JBLK : (ih + 1) * JBLK],
                            start=(dd == 0),
                            stop=(dd == DO - 1),
                        )
                for ih in range(R // JBLK):
                    nc.scalar.activation(
                        ptb[:, jj, ih * JBLK : (ih + 1) * JBLK],
                        pe_h[ih],
                        AF.Exp,
                        scale=SCALE,
                    )
            # out_unnorm += P^T.T @ V, with the exp-sums matmul sharing each
            # stationary ptb tile
            ps = s_psum.tile([P, 2 * IC], F32, tag="ps")
            for ic in range(IC):
                po_h = [
                    o_psum.tile([P, 512], F32, tag="po", name="po") for _ in range(2)
                ]
                for jj in range(JBLK // P):
                    for oh in range(2):
                        nc.tensor.matmul(
                            po_h[oh],
                            ptb[:, jj, ic * P : (ic + 1) * P],
                            vtb[:, jj, oh * 512 : (oh + 1) * 512],
                            start=(jj == 0),
                            stop=(jj == JBLK // P - 1),
                        )
                    nc.tensor.matmul(
                        ps[:, 2 * ic : 2 * ic + 2],
                        ptb[:, jj, ic * P : (ic + 1) * P],
                        ones,
                        start=(ic == 0 and jj == 0),
                        stop=(ic == IC - 1 and jj == JBLK // P - 1),
                    )
                for oh in range(2):
                    dst = outacc[:, ic, oh * 512 : (oh + 1) * 512]
                    if jb == 0:
                        nc.vector.tensor_copy(out=dst, in_=po_h[oh])
                    else:
                        nc.vector.tensor_tensor(dst, po_h[oh], dst, ALU.add)
            if jb == 0:
                nc.vector.tensor_copy(out=sums_acc, in_=ps)
            else:
                nc.vector.tensor_tensor(sums_acc, ps, sums_acc, ALU.add)

        # ---- epilogue: normalize, add bv, write out ----
        nc.vector.reciprocal(rsum, sums_acc)
        fin_pool = p2.enter_context(tc.tile_pool(name="fin", bufs=2))
        for ic in range(IC):
            ofin = fin_pool.tile([P, D], F32, tag="ofin")
            nc.vector.tensor_scalar_mul(
                ofin, outacc[:, ic, :], rsum[:, 2 * ic : 2 * ic + 1]
            )
            nc.vector.tensor_tensor(ofin, ofin, bv_bc, ALU.add)
            nc.sync.dma_start(out_loc[ic * P : (ic + 1) * P, :], ofin)

    outer.close()


_NC_CACHE = None


def _get_program():
    global _NC_CACHE
    if _NC_CACHE is None:
        _NC_CACHE = build_program()
    return _NC_CACHE


def _run(inputs, trace=False):
    nc = _get_program()
    x = np.ascontiguousarray(np.asarray(inputs["x"], dtype=np.float32))
    common = {
        k: np.ascontiguousarray(np.asarray(inputs[k], dtype=np.float32))
        for k in ("Wq", "Wk", "Wv", "bq", "bv")
    }
    in_maps = [
        {"x_loc": np.ascontiguousarray(x[c * R : (c + 1) * R]), **common}
        for c in range(NCORES)
    ]
    res = run_bass_kernel_spmd(
        nc, in_maps, core_ids=list(range(NCORES)), trace=trace
    )
    out = np.concatenate([res.results[c]["out_loc"] for c in range(NCORES)], axis=0)
    return out.reshape(B, D, 1).astype(np.float32), res


def kernel(**inputs):
    out, _ = _run(inputs, trace=False)
    return out
